# revision 3
# baseline (speedup 1.0000x reference)
"""Trainium2 Bass kernel for nn_DiffHist (differentiable 256-bin histogram).

Contract: kernel(img) takes the FULL input img [128, 512, 512] f32 with
values in [0, 1], returns the FULL output h[256] f32 — identical math to
the reference:
    s = 255*img.ravel(); idx = floor(s); d = s - idx
    h[idx] += 1-d; h[idx+1] += d; return h[:256]

Strategy (data-parallel over 8 NeuronCores; each core gets 1/8 of the
flattened image as a [128, 32768] f32 block):

  Per core, the histogram is computed as a PSUM-accumulated bilinear
  form on the tensor engine.  With u = s/16 in [0, 16), coarse block
  a = floor(u) (16 blocks of 16 bins) and fine offset lo = 16*frac(u):

      h[16a + b] = sum_i [a_i == a] * tent(lo_i - b),  b = 0..16
      tent(d) = relu(1 - |d|) = relu(d+1) - 2 relu(d) + relu(d-1)

  Each chunk of 128 elements (one SBUF column) contributes one
  rank-128 update:  lhsT = U[k, a] = [a_k == a] (one-hot, 16 cols),
  rhs = V[k, p] = relu(lo_k - (p-1)) (ramp columns c = -1..17).  G=8
  chunks are packed per matmul (block-diagonal), so each matmul is
  lhsT [128, 128] x rhs [128, 152] accumulated into one PSUM tile; the tent
  second difference and the block-diagonal extraction happen on the
  host at gather time, as does the 8-way sum (the all-reduce of the
  per-core 272-float partial histograms).

  floor/frac are built with the fp32 magic-number trick
  (R = (u - 0.5) + 1.5*2^23) since the DVE has no floor/mod ALU op.

Numerics: U is exact {0,1}; lo is fp16 (|err| <= 2^-7 bin units) and V
ramps are fp16; PSUM accumulates in fp32.  Measured end-to-end relative
L2 error vs the fp64 reference is ~2e-5.
"""
import sys

sys.path.insert(0, '/opt/trn_rl_repo')

import numpy as np

# ----------------------------------------------------------------- tile patch
# The pinned walrus build accepts only one sync-wait command on several
# instruction classes; current concourse Tile attaches several to the
# kernel-tail drain and occasionally to DMA ops.  Split the excess waits
# onto dedicated single-wait instructions.
import bass_rust
import concourse.tile as tile
import concourse.mybir as mybir
from bass_rust import ScopedClock

_MAX_WAITS = 1


def _drain_and_barrier_split(self, tick_clock, wait_clock):
    nc = self.nc
    drain_inst = nc.sync.drain()
    wait_clock.add_sem_waits(
        drain_inst.ins, ScopedClock({None: tick_clock.global_clock})
    )
    si = drain_inst.ins.sync_info
    waits = list(si.on_wait) if si is not None and si.on_wait else []
    if len(waits) > _MAX_WAITS:
        drain_inst.ins.sync_info = bass_rust.SyncInfo(
            on_wait=waits[:_MAX_WAITS], on_update=list(si.on_update)
        )
        for w in waits[_MAX_WAITS:]:
            d2 = nc.sync.drain()
            d2.ins.sync_info = bass_rust.SyncInfo(on_wait=[w], on_update=[])
    nc.all_engine_barrier()
    assert self.sems is not None
    popped = nc._tile_sem_poison_stack.pop()
    assert popped is self._sem_poison
    nc.clear_and_free_semaphores(list(self.sems.allocated().values()))
    nc.all_engine_barrier()


def _split_excess_waits(nc, max_waits=_MAX_WAITS):
    for bb in nc.main_func.blocks:
        insts = list(bb.instructions)
        out = []
        changed = False
        for ins in insts:
            si = ins.sync_info
            if si is not None and si.on_wait and len(si.on_wait) > max_waits:
                waits = list(si.on_wait)
                extra, keep = waits[:-max_waits], waits[-max_waits:]
                for w in extra:
                    nop = mybir.InstNoOp(
                        name=f"waitnop-{nc.next_id()}",
                        engine=ins.engine,
                        bass_nofuse=True,
                        sync_info=mybir.SyncInfo(on_wait=[w], on_update=[]),
                    )
                    nc.register_instruction(nop, overwrite=True)
                    out.append(nop)
                ins.sync_info = bass_rust.SyncInfo(
                    on_wait=keep, on_update=list(si.on_update)
                )
                changed = True
            out.append(ins)
        if changed:
            bb.instructions = out


tile.TileContext._drain_and_barrier = _drain_and_barrier_split

# ----------------------------------------------------------------- kernel
import concourse.bass as bass

F32 = mybir.dt.float32
F16 = mybir.dt.float16
ALU = mybir.AluOpType
ACTF = mybir.ActivationFunctionType

NCORES = 8
NCOLS = 32768          # elements per partition per core
NA = 16                # coarse blocks
NB = 19                # relu ramp columns c = -1..17 (tent = 2nd diff)
G = 8                  # chunks per matmul
NOUT = NB * G          # 152
FD = 1024              # columns per tile
MAGIC = 12582912.0     # 1.5 * 2^23


def _build_nc():
    nc = bass.Bass()
    x = nc.declare_dram_parameter("x", [128, NCOLS], F32, isOutput=False)
    out = nc.declare_dram_parameter("hist", [128, NOUT], F32, isOutput=True)
    ntiles = NCOLS // FD

    with tile.TileContext(nc) as tc:
        with (
            tc.tile_pool(name="sb", bufs=2) as sb,
            tc.tile_pool(name="sbo", bufs=1) as sbo,
            tc.tile_pool(name="psum", bufs=1, space="PSUM") as psum,
        ):
            acc = psum.tile([128, NOUT], F32)
            for t in range(ntiles):
                xt = sb.tile([128, FD], F32, tag="x")
                nc.sync.dma_start(xt[:], x[:, t * FD:(t + 1) * FD])
                u = sb.tile([128, FD], F32, tag="u")
                R = sb.tile([128, FD], F32, tag="R")
                negf = sb.tile([128, FD], F32, tag="negf")
                lo = sb.tile([128, FD], F16, tag="lo")
                hiF = sb.tile([128, FD], F16, tag="hi")
                # u = x*(255/16) in [0,16); fp32 magic-number floor:
                # R = (u - 0.5) + 1.5*2^23 -> R - MAGIC = floorish(u)
                # (round-half-even at exact integers is absorbed by the
                # tent overlap column)
                nc.vector.tensor_scalar(u[:], xt[:], 255.0 / 16.0, None,
                                        ALU.mult)
                nc.vector.tensor_scalar(R[:], u[:], -0.5, MAGIC, ALU.add,
                                        ALU.add)
                nc.vector.scalar_tensor_tensor(
                    negf[:], R[:], -MAGIC, u[:], ALU.add, ALU.subtract)
                nc.vector.tensor_scalar(lo[:], negf[:], -16.0, None, ALU.mult)
                nc.vector.tensor_scalar(hiF[:], R[:], -MAGIC, None, ALU.add)
                U = sb.tile([128, FD // G, NA, G], F16, tag="U")
                V = sb.tile([128, FD // G, NB, G], F16, tag="V")
                hiG = hiF[:].rearrange("p (q g) -> p q g", g=G)
                loG = lo[:].rearrange("p (q g) -> p q g", g=G)
                for a in range(NA):
                    nc.vector.tensor_scalar(
                        U[:, :, a, :], hiG, float(a), None, ALU.is_equal)
                for p in range(NB):
                    # ramp column c = p-1: relu(lo - c); tent recovered at
                    # readout via tent(d) = relu(d+1) - 2 relu(d) + relu(d-1)
                    nc.vector.tensor_scalar(
                        V[:, :, p, :], loG, float(p - 1), 0.0,
                        ALU.subtract, ALU.max)
                for q in range(FD // G):
                    nc.tensor.matmul(
                        acc[:],
                        U[:, q].opt(),
                        V[:, q].opt(),
                        start=(t == 0 and q == 0),
                        stop=(t == ntiles - 1 and q == FD // G - 1),
                    )
            res = sbo.tile([128, NOUT], F32)
            nc.vector.tensor_copy(res[:], acc[:])
            nc.sync.dma_start(out[:], res[:])
    _split_excess_waits(nc)
    return nc


_NC_CACHE = None


def _get_nc():
    global _NC_CACHE
    if _NC_CACHE is None:
        _NC_CACHE = _build_nc()
    return _NC_CACHE


def _shard(img):
    flat = np.ascontiguousarray(np.asarray(img, dtype=np.float32)).reshape(-1)
    assert flat.size == NCORES * 128 * NCOLS
    return flat.reshape(NCORES, 128, NCOLS)


def _combine(per_core_hists):
    P = np.zeros((128, NOUT), np.float64)
    for r in per_core_hists:
        P += np.asarray(r, dtype=np.float64)
    R = P.reshape(NA, G, NB, G)
    CR = np.einsum('agbg->ab', R)          # [16, 19] ramp sums, c=-1..17
    T = CR[:, 0:17] - 2.0 * CR[:, 1:18] + CR[:, 2:19]   # tent sums b=0..16
    h = np.zeros(NA * 16 + 1, np.float64)
    for a in range(NA):
        h[16 * a:16 * a + 16] += T[a, :16]
        h[16 * a + 16] += T[a, 16]
    return h[:256].astype(np.float32)


def kernel(img):
    from concourse.bass_utils import run_bass_kernel_spmd
    shards = _shard(img)
    in_maps = [{"x": shards[i]} for i in range(NCORES)]
    res = run_bass_kernel_spmd(_get_nc(), in_maps, core_ids=list(range(NCORES)))
    return _combine([res.results[i]["hist"] for i in range(NCORES)])
